# revision 12
# baseline (speedup 1.0000x reference)
"""CapsNet forward on 8 trn2 NeuronCores — data-parallel convs on device."""
import numpy as np
import ml_dtypes

B = 256
NCORES = 8
BL = B // NCORES          # 32 images per core
POS1 = 32 * 20 * 20       # conv1 output positions per core (img,oh,ow)
K1 = 82                   # 81 taps + 1 bias row
KHW = 81
NPOS2 = 36                # 6x6
CHUNKS = [(0, 12), (12, 12), (24, 8)]

_exec_time_ns = None


def _dedupe_ldweights(nc, mybir):
    """Drop InstLdweights that reload the exact weights already resident in
    the PE array (identical consecutive loads with no clobber between)."""
    removed = 0
    for f in nc.m.functions:
        for b in f.blocks:
            il = b.instructions
            last_key = None
            to_del = []
            for idx, i in enumerate(il):
                tn = type(i).__name__
                if tn == 'InstLdweights':
                    ap = i.ins[0]
                    key = (str(ap.memref), ap.offset, str(ap.ap), str(ap.dtype))
                    if key == last_key:
                        to_del.append(idx)
                    last_key = key
                elif tn == 'InstMatmult':
                    if i.is_transpose:
                        last_key = None
                elif getattr(i, 'engine', None) == mybir.EngineType.PE:
                    last_key = None
            for idx in reversed(to_del):
                del il[idx]
            removed += len(to_del)
    return removed


def _build_and_run_device(im2col_np, w1t_np, w2_np):
    import concourse.bass as bass
    import concourse.bacc as bacc
    import concourse.mybir as mybir
    import concourse.tile as tile

    bf16 = mybir.dt.bfloat16
    f32 = mybir.dt.float32
    AF = mybir.ActivationFunctionType

    nc = bacc.Bacc("TRN2", target_bir_lowering=False, debug=False,
                   enable_asserts=False, num_devices=NCORES)
    # im2col with w1t packed as trailing columns: each SBUF partition row is
    # one contiguous ~26KB descriptor (HBM->SBUF descriptors cost ~0.5us
    # apiece regardless of size, so fewer+bigger descriptors spread over
    # more parallel dma_starts win)
    EXT = POS1 + 256
    imx_d = nc.dram_tensor("imx", [K1, EXT], bf16, kind="ExternalInput")
    w2_d = nc.dram_tensor("w2", [2, 128, KHW * 256], bf16, kind="ExternalInput")
    uout_d = nc.dram_tensor("uout", [2, 128, BL * NPOS2], f32, kind="ExternalOutput")

    with tile.TileContext(nc) as tc:
        with tc.tile_pool(name="const", bufs=1) as const, \
             tc.tile_pool(name="ps1", bufs=2, space="PSUM") as ps1, \
             tc.tile_pool(name="ps2", bufs=2, space="PSUM") as ps2, \
             tc.tile_pool(name="outp", bufs=2) as outp:
            # inputs for conv1 first so its compute starts ASAP
            imx_sb = const.tile([K1, EXT], bf16, name="imx_sb")
            bnds = [K1 * g // 8 for g in range(9)]
            for g in range(8):
                nc.sync.dma_start(imx_sb[bnds[g]:bnds[g + 1], :],
                                  imx_d.ap()[bnds[g]:bnds[g + 1], :])
            im2col_sb = imx_sb[:, :POS1]
            w1t_sb = imx_sb[:, POS1:]
            # w2: 2 x 8 parallel row-group dma_starts, 41.5KB descriptors
            w2_sb = []
            for ci in range(2):
                t = const.tile([128, KHW * 256], bf16, name=f"w2_{ci}")
                for g in range(8):
                    nc.sync.dma_start(t[g * 16:(g + 1) * 16, :],
                                      w2_d.ap()[ci][g * 16:(g + 1) * 16, :])
                w2_sb.append(t)
            x1 = [const.tile([128, POS1], bf16, name=f"x1_{ot}")
                  for ot in range(2)]

            # conv1 + relu: out[oc, (img,oh,ow)] = relu(W1.T @ im2col)
            # relu alternates between Scalar and Vector engines so the
            # PSUM drain keeps pace with the PE
            for ot in range(2):
                for c in range(POS1 // 512):
                    ps = ps1.tile([128, 512], f32, tag="c1", name=f"c1_{ot}_{c}")
                    nc.tensor.matmul(
                        ps[:], w1t_sb[:, ot * 128:(ot + 1) * 128],
                        im2col_sb[:, c * 512:(c + 1) * 512],
                        start=True, stop=True)
                    dst = x1[ot][:, c * 512:(c + 1) * 512]
                    if c % 2 == 0:
                        nc.scalar.activation(dst, ps[:], AF.Relu)
                    else:
                        nc.vector.tensor_relu(dst, ps[:])

            # primary caps conv: stride 2, 9x9, 256->256; 162 acc steps
            x1v = [x1[ci][:].rearrange("p (b h w) -> p b h w", b=BL, h=20, w=20)
                   for ci in range(2)]
            for ot in range(2):
                pss = []
                for ic, (b0, nb) in enumerate(CHUNKS):
                    pss.append(ps2.tile([128, nb * NPOS2], f32, tag=f"c2_{ic}",
                                        name=f"c2_{ot}_{ic}"))
                nk = 0
                for ci in range(2):
                    for kh in range(9):
                        for kw in range(9):
                            khkw = kh * 9 + kw
                            lhsT = w2_sb[ci][:, khkw * 256 + ot * 128:
                                             khkw * 256 + ot * 128 + 128]
                            for ic, (b0, nb) in enumerate(CHUNKS):
                                rhs = x1v[ci][:, b0:b0 + nb,
                                              kh:kh + 11:2, kw:kw + 11:2]
                                nc.tensor.matmul(pss[ic][:], lhsT, rhs,
                                                 start=(nk == 0), stop=(nk == 161))
                            nk += 1
                stage = outp.tile([128, BL * NPOS2], f32, tag="st",
                                  name=f"st_{ot}")
                for ic, (b0, nb) in enumerate(CHUNKS):
                    nc.scalar.activation(
                        stage[:, b0 * NPOS2:(b0 + nb) * NPOS2], pss[ic][:],
                        AF.Copy)
                nc.sync.dma_start(uout_d.ap()[ot], stage[:])

    nldw = _dedupe_ldweights(nc, mybir)
    print("dedupe removed", nldw, "ldweights")
    nc.compile()
    in_maps = [{"imx": im2col_np[c], "w2": w2_np}
               for c in range(NCORES)]
    res, t_ns = _run_profiled(nc, in_maps)
    global _exec_time_ns
    _exec_time_ns = t_ns
    return [res.results[c]["uout"].astype(np.float32) for c in range(NCORES)]


def _run_profiled(nc, in_maps):
    """Execute on all 8 cores; NTFF-profile core 0 to get true HW exec time."""
    import os, tempfile
    from concourse.bass_utils import run_bass_kernel_spmd

    core_ids = list(range(NCORES))
    neff_dir = tempfile.mkdtemp(prefix="ntff_prof_")
    hook = None
    try:
        from trn_agent_boot.trn_boot import _ntff_profile_via_ctypes
        hook = _ntff_profile_via_ctypes('/opt/axon/libaxon_pjrt.so')
    except Exception:
        hook = None

    if hook is None:
        return run_bass_kernel_spmd(nc, in_maps, core_ids=core_ids), None

    try:
        with hook(neff_dir, [0]):
            res = run_bass_kernel_spmd(nc, in_maps, core_ids=core_ids)
    except Exception:
        import traceback
        traceback.print_exc()
        return run_bass_kernel_spmd(nc, in_maps, core_ids=core_ids), None

    t_ns = None
    try:
        import gauge.profiler
        from gauge import trn_perfetto
        from concourse._compat import FishPath
        profile = gauge.profiler.Profile(
            profile_path=FishPath(neff_dir), kernel_dev_mode=True,
            profile_on_exit=False, bass_kernel=nc.m,
            offline_processing=True, fname="*_body*")
        profile.convert_ntffs_to_json((0,))
        json_path = profile.json_path(0).path
        _, _, t_ns, _ = trn_perfetto.main(
            json=json_path, kernel_dev_mode=True, bass_kernel=nc.m,
            out_path=os.path.join(neff_dir, "trace.pftrace"))
        print("TRACE DIR:", neff_dir, "exec_time_ns:", t_ns)
    except Exception:
        import traceback
        traceback.print_exc()
    return res, t_ns


def _host_conv_fallback(im2col_np, w1t_np, w2_np):
    outs = []
    for c in range(NCORES):
        af = im2col_np[c].astype(np.float32)         # [82, POS1+256]
        a, w1 = af[:, :POS1], af[:, POS1:]
        x1 = np.maximum(w1.T @ a, 0.0)               # [256, POS1]
        x1 = x1.reshape(2, 128, BL, 20, 20)
        w2 = w2_np.astype(np.float32).reshape(2, 128, KHW, 256)
        acc = np.zeros((256, BL * NPOS2), np.float32)
        for kh in range(9):
            for kw in range(9):
                khkw = kh * 9 + kw
                for ci in range(2):
                    patch = x1[ci, :, :, kh:kh + 11:2, kw:kw + 11:2] \
                        .reshape(128, -1)
                    acc += w2[ci, :, khkw, :].T @ patch
        outs.append(acc.reshape(2, 128, BL * NPOS2))
    return outs


def kernel(images, labels, conv1_w, conv1_b, prim_w, prim_b, W):
    images = np.asarray(images, np.float32)
    conv1_w = np.asarray(conv1_w, np.float32)
    conv1_b = np.asarray(conv1_b, np.float32)
    prim_w = np.asarray(prim_w, np.float32)
    prim_b = np.asarray(prim_b, np.float32)
    W = np.asarray(W, np.float32)

    # host staging: im2col per core with w1t packed as trailing columns
    w1t = np.concatenate([conv1_w.reshape(256, KHW).T, conv1_b[None, :]], 0)
    im2col_np = []
    for c in range(NCORES):
        img = images[c * BL:(c + 1) * BL, 0]                   # [32,28,28]
        sw = np.lib.stride_tricks.sliding_window_view(img, (9, 9), axis=(1, 2))
        a = sw.transpose(3, 4, 0, 1, 2).reshape(KHW, POS1)     # [81, POS1]
        a = np.concatenate([a, np.ones((1, POS1), np.float32)], 0)
        a = np.concatenate([a, w1t], 1)                        # [82, POS1+256]
        im2col_np.append(np.ascontiguousarray(a).astype(ml_dtypes.bfloat16))
    w1t_np = None
    # [i, khkw, o] packed as [ci, ic_local, khkw*256 + oc]
    w2_np = prim_w.reshape(256, 256, KHW).transpose(1, 2, 0) \
        .reshape(2, 128, KHW * 256).astype(ml_dtypes.bfloat16)

    try:
        uouts = _build_and_run_device(im2col_np, w1t_np, w2_np)
    except Exception as e:
        import traceback
        traceback.print_exc()
        print("DEVICE PATH FAILED — numpy fallback:", e)
        uouts = _host_conv_fallback(im2col_np, w1t_np, w2_np)

    # host epilogue (exact reference math, f32)
    us = []
    for c in range(NCORES):
        y = uouts[c].reshape(256, BL, NPOS2) + prim_b[:, None, None]
        u = y.reshape(8, 32, BL, NPOS2).transpose(2, 0, 1, 3).reshape(BL, 8, 1152)
        us.append(u)
    u = np.concatenate(us, 0).transpose(0, 2, 1)               # [B,1152,8]

    sq = np.sum(u * u, axis=1, keepdims=True)                  # [B,1,8]
    u = sq / (1.0 + sq) * (u / np.sqrt(sq))
    # u_hat[b,r,j,d]
    u_hat = np.einsum('rjdi,bri->brjd', W, u, optimize=True).astype(np.float32)
    b_ij = np.zeros((1152, 10), np.float32)
    for _ in range(3):
        e = np.exp(b_ij - b_ij.max(axis=1, keepdims=True))
        c_ij = e / e.sum(axis=1, keepdims=True)
        s_j = np.einsum('rj,brjd->bjd', c_ij, u_hat, optimize=True)
        sq2 = np.sum(s_j * s_j, axis=2, keepdims=True)
        v_j = sq2 / (1.0 + sq2) * (s_j / np.sqrt(sq2))
        agree = np.einsum('brjd,bjd->brj', u_hat, v_j, optimize=True).mean(axis=0)
        b_ij = b_ij + agree
    return v_j[..., None].astype(np.float32)


# revision 25
# speedup vs baseline: 1.3801x; 1.3801x over previous
"""CapsNet forward on 8 trn2 NeuronCores — data-parallel convs on device."""
import numpy as np
import ml_dtypes

B = 256
NCORES = 8
BL = B // NCORES          # 32 images per core
POS1 = 32 * 20 * 20       # conv1 output positions per core (img,oh,ow)
K1 = 82                   # 81 taps + 1 bias row
KHW = 81
NPOS2 = 36                # 6x6
CHUNKS = [(0, 12), (12, 12), (24, 8)]

_exec_time_ns = None


def _dedupe_ldweights(nc, mybir):
    """Drop InstLdweights that reload the exact weights already resident in
    the PE array (identical consecutive loads with no clobber between)."""
    removed = 0
    for f in nc.m.functions:
        for b in f.blocks:
            il = b.instructions
            last_key = None
            to_del = []
            for idx, i in enumerate(il):
                tn = type(i).__name__
                if tn == 'InstLdweights':
                    ap = i.ins[0]
                    key = (str(ap.memref), ap.offset, str(ap.ap), str(ap.dtype))
                    if key == last_key:
                        to_del.append(idx)
                    last_key = key
                elif tn == 'InstMatmult':
                    if i.is_transpose:
                        last_key = None
                elif getattr(i, 'engine', None) == mybir.EngineType.PE:
                    last_key = None
            for idx in reversed(to_del):
                del il[idx]
            removed += len(to_del)
    return removed


def _build_and_run_device(im2col_np, w1t_np, w2_np):
    import concourse.bass as bass
    import concourse.bacc as bacc
    import concourse.mybir as mybir
    import concourse.tile as tile

    bf16 = mybir.dt.bfloat16
    f32 = mybir.dt.float32
    AF = mybir.ActivationFunctionType

    nc = bacc.Bacc("TRN2", target_bir_lowering=False, debug=False,
                   enable_asserts=False, num_devices=NCORES)
    # im2col with w1t packed as trailing columns: each SBUF partition row is
    # one contiguous ~26KB descriptor (HBM->SBUF descriptors cost ~0.5us
    # apiece regardless of size, so fewer+bigger descriptors spread over
    # more parallel dma_starts win)
    EXT = POS1 + 256
    imx_d = nc.dram_tensor("imx", [128, EXT], bf16, kind="ExternalInput")
    w2_d = nc.dram_tensor("w2", [2, 128, KHW * 256], bf16, kind="ExternalInput")
    uout_d = nc.dram_tensor("uout", [2, 128, BL * NPOS2], f32, kind="ExternalOutput")

    with tile.TileContext(nc) as tc:
        with tc.tile_pool(name="const", bufs=1) as const, \
             tc.tile_pool(name="ps1", bufs=5, space="PSUM") as ps1, \
             tc.tile_pool(name="ps2", bufs=1, space="PSUM") as ps2, \
             tc.tile_pool(name="outp", bufs=2) as outp:
            # conv1 inputs first. The tensor is zero-padded to 128 partition
            # rows: a full-128-partition destination spreads its descriptors
            # across all 16 DMA engines, while partial-partition slices pin
            # to one engine (~8GB/s)
            imx_sb = const.tile([128, EXT], bf16, name="imx_sb")
            nc.sync.dma_start(imx_sb[:], imx_d.ap()[:, :])
            im2col_sb = imx_sb[0:K1, :POS1]
            w1t_sb = imx_sb[0:K1, POS1:]
            # blocker: a tiny DMA that reads a sliver of imx, so the sync
            # engine stalls here until imx fully lands and the w2 descriptors
            # below can't compete with imx for DMA bandwidth
            blk = const.tile([16, 64], bf16, name="blk")
            nc.sync.dma_start(blk[:], imx_sb[0:128:8, 0:64])
            # w2 streamed as (ci, kh) column chunks in consumption order;
            # each 128-row start spreads across all 16 engines and the
            # per-engine FIFOs preserve issue order
            w2_sb = []
            for ci in range(2):
                t = const.tile([128, KHW * 256], bf16, name=f"w2_{ci}")
                w2_sb.append(t)
            for ci in range(2):
                for kh in range(9):
                    c0, c1 = kh * 9 * 256, (kh + 1) * 9 * 256
                    nc.sync.dma_start(w2_sb[ci][:, c0:c1],
                                      w2_d.ap()[ci][:, c0:c1])
            x1 = [const.tile([128, POS1], bf16, name=f"x1_{ot}")
                  for ot in range(2)]

            # conv1 + relu: out[oc, (img,oh,ow)] = relu(W1.T @ im2col)
            # relu alternates between Scalar and Vector engines so the
            # PSUM drain keeps pace with the PE
            for ot in range(2):
                for c in range(POS1 // 512):
                    ps = ps1.tile([128, 512], f32, tag="c1", name=f"c1_{ot}_{c}")
                    nc.tensor.matmul(
                        ps[:], w1t_sb[:, ot * 128:(ot + 1) * 128],
                        im2col_sb[:, c * 512:(c + 1) * 512],
                        start=True, stop=True)
                    dst = x1[ot][:, c * 512:(c + 1) * 512]
                    if c % 2 == 0:
                        nc.scalar.activation(dst, ps[:], AF.Relu)
                    else:
                        nc.vector.tensor_relu(dst, ps[:])

            # primary caps conv: stride 2, 9x9, 256->256; 162 acc steps
            x1v = [x1[ci][:].rearrange("p (b h w) -> p b h w", b=BL, h=20, w=20)
                   for ci in range(2)]
            for ot in range(2):
                pss = []
                for ic, (b0, nb) in enumerate(CHUNKS):
                    pss.append(ps2.tile([128, nb * NPOS2], f32, tag=f"c2_{ic}",
                                        name=f"c2_{ot}_{ic}"))
                nk = 0
                for ci in range(2):
                    for kh in range(9):
                        for kw in range(9):
                            khkw = kh * 9 + kw
                            lhsT = w2_sb[ci][:, khkw * 256 + ot * 128:
                                             khkw * 256 + ot * 128 + 128]
                            for ic, (b0, nb) in enumerate(CHUNKS):
                                rhs = x1v[ci][:, b0:b0 + nb,
                                              kh:kh + 11:2, kw:kw + 11:2]
                                nc.tensor.matmul(pss[ic][:], lhsT, rhs,
                                                 start=(nk == 0), stop=(nk == 161))
                            nk += 1
                stage = outp.tile([128, BL * NPOS2], f32, tag="st",
                                  name=f"st_{ot}")
                for ic, (b0, nb) in enumerate(CHUNKS):
                    nc.scalar.activation(
                        stage[:, b0 * NPOS2:(b0 + nb) * NPOS2], pss[ic][:],
                        AF.Copy)
                nc.sync.dma_start(uout_d.ap()[ot], stage[:])

    nldw = _dedupe_ldweights(nc, mybir)
    print("dedupe removed", nldw, "ldweights")
    nc.compile()
    in_maps = [{"imx": im2col_np[c], "w2": w2_np}
               for c in range(NCORES)]
    res, t_ns = _run_profiled(nc, in_maps)
    global _exec_time_ns
    _exec_time_ns = t_ns
    return [res.results[c]["uout"].astype(np.float32) for c in range(NCORES)]


def _run_profiled(nc, in_maps):
    """Execute on all 8 cores; NTFF-profile core 0 to get true HW exec time."""
    import os, tempfile
    from concourse.bass_utils import run_bass_kernel_spmd

    core_ids = list(range(NCORES))
    neff_dir = tempfile.mkdtemp(prefix="ntff_prof_")
    hook = None
    try:
        from trn_agent_boot.trn_boot import _ntff_profile_via_ctypes
        hook = _ntff_profile_via_ctypes('/opt/axon/libaxon_pjrt.so')
    except Exception:
        hook = None

    if hook is None:
        return run_bass_kernel_spmd(nc, in_maps, core_ids=core_ids), None

    try:
        with hook(neff_dir, [0]):
            res = run_bass_kernel_spmd(nc, in_maps, core_ids=core_ids)
    except Exception:
        import traceback
        traceback.print_exc()
        return run_bass_kernel_spmd(nc, in_maps, core_ids=core_ids), None

    t_ns = None
    try:
        import gauge.profiler
        from gauge import trn_perfetto
        from concourse._compat import FishPath
        profile = gauge.profiler.Profile(
            profile_path=FishPath(neff_dir), kernel_dev_mode=True,
            profile_on_exit=False, bass_kernel=nc.m,
            offline_processing=True, fname="*_body*")
        profile.convert_ntffs_to_json((0,))
        json_path = profile.json_path(0).path
        _, _, t_ns, _ = trn_perfetto.main(
            json=json_path, kernel_dev_mode=True, bass_kernel=nc.m,
            out_path=os.path.join(neff_dir, "trace.pftrace"))
        print("TRACE DIR:", neff_dir, "exec_time_ns:", t_ns)
    except Exception:
        import traceback
        traceback.print_exc()
    return res, t_ns


def _host_conv_fallback(im2col_np, w1t_np, w2_np):
    outs = []
    for c in range(NCORES):
        af = im2col_np[c][:K1].astype(np.float32)    # [82, POS1+256]
        a, w1 = af[:, :POS1], af[:, POS1:]
        x1 = np.maximum(w1.T @ a, 0.0)               # [256, POS1]
        x1 = x1.reshape(2, 128, BL, 20, 20)
        w2 = w2_np.astype(np.float32).reshape(2, 128, KHW, 256)
        acc = np.zeros((256, BL * NPOS2), np.float32)
        for kh in range(9):
            for kw in range(9):
                khkw = kh * 9 + kw
                for ci in range(2):
                    patch = x1[ci, :, :, kh:kh + 11:2, kw:kw + 11:2] \
                        .reshape(128, -1)
                    acc += w2[ci, :, khkw, :].T @ patch
        outs.append(acc.reshape(2, 128, BL * NPOS2))
    return outs


def kernel(images, labels, conv1_w, conv1_b, prim_w, prim_b, W):
    images = np.asarray(images, np.float32)
    conv1_w = np.asarray(conv1_w, np.float32)
    conv1_b = np.asarray(conv1_b, np.float32)
    prim_w = np.asarray(prim_w, np.float32)
    prim_b = np.asarray(prim_b, np.float32)
    W = np.asarray(W, np.float32)

    # host staging: im2col per core with w1t packed as trailing columns
    w1t = np.concatenate([conv1_w.reshape(256, KHW).T, conv1_b[None, :]], 0)
    im2col_np = []
    for c in range(NCORES):
        img = images[c * BL:(c + 1) * BL, 0]                   # [32,28,28]
        sw = np.lib.stride_tricks.sliding_window_view(img, (9, 9), axis=(1, 2))
        a = sw.transpose(3, 4, 0, 1, 2).reshape(KHW, POS1)     # [81, POS1]
        a = np.concatenate([a, np.ones((1, POS1), np.float32)], 0)
        a = np.concatenate([a, w1t], 1)                        # [82, POS1+256]
        a = np.concatenate([a, np.zeros((128 - K1, POS1 + 256), np.float32)], 0)
        im2col_np.append(np.ascontiguousarray(a).astype(ml_dtypes.bfloat16))
    w1t_np = None
    # [i, khkw, o] packed as [ci, ic_local, khkw*256 + oc]
    w2_np = prim_w.reshape(256, 256, KHW).transpose(1, 2, 0) \
        .reshape(2, 128, KHW * 256).astype(ml_dtypes.bfloat16)

    try:
        uouts = _build_and_run_device(im2col_np, w1t_np, w2_np)
    except Exception as e:
        import traceback
        traceback.print_exc()
        print("DEVICE PATH FAILED — numpy fallback:", e)
        uouts = _host_conv_fallback(im2col_np, w1t_np, w2_np)

    # host epilogue (exact reference math, f32)
    us = []
    for c in range(NCORES):
        y = uouts[c].reshape(256, BL, NPOS2) + prim_b[:, None, None]
        u = y.reshape(8, 32, BL, NPOS2).transpose(2, 0, 1, 3).reshape(BL, 8, 1152)
        us.append(u)
    u = np.concatenate(us, 0).transpose(0, 2, 1)               # [B,1152,8]

    sq = np.sum(u * u, axis=1, keepdims=True)                  # [B,1,8]
    u = sq / (1.0 + sq) * (u / np.sqrt(sq))
    # u_hat[b,r,j,d]
    u_hat = np.einsum('rjdi,bri->brjd', W, u, optimize=True).astype(np.float32)
    b_ij = np.zeros((1152, 10), np.float32)
    for _ in range(3):
        e = np.exp(b_ij - b_ij.max(axis=1, keepdims=True))
        c_ij = e / e.sum(axis=1, keepdims=True)
        s_j = np.einsum('rj,brjd->bjd', c_ij, u_hat, optimize=True)
        sq2 = np.sum(s_j * s_j, axis=2, keepdims=True)
        v_j = sq2 / (1.0 + sq2) * (s_j / np.sqrt(sq2))
        agree = np.einsum('brjd,bjd->brj', u_hat, v_j, optimize=True).mean(axis=0)
        b_ij = b_ij + agree
    return v_j[..., None].astype(np.float32)
